# revision 8
# baseline (speedup 1.0000x reference)
"""Causal GQA self-attention (B=4, T=2048, C=2048, 16 heads / 4 kv-heads,
l2-normalized q,k) on 8 Trainium2 NeuronCores.

Sharding: core pair (2b, 2b+1) handles batch b. Within a pair, queries are
split stride-2 by row parity (parity p takes rows p::2), which balances the
causal work and keeps the compiled program identical on all 8 cores. All
per-core differences are data: odd-parity cores receive x with adjacent
rows pair-swapped, so the program's fixed `rows 0::2` query gather selects
the odd rows while every key row remains present; the swapped in-chunk key
order is absorbed into that core's causal-mask tiles (a kernel input).

Per core:
  xT      = PE-transpose(x_b) in bf16, plus an fp8(e4m3) copy
  q,k     = fp8 DoubleRow-mode projections (2 contraction tiles per pass,
            0.5 cyc/row); weights pre-scaled by 64 on host to escape fp8
            subnormals -- the subsequent l2norm makes q,k scale-invariant
  v       = bf16 projection of full T, re-transposed on PE
  q rows  = this core's 1024 query rows extracted strided from the fp8 xT
            into SBUF (no DRAM bounce)
  attn    = scores^T layout sT[k,q] (bf16), exp on ACT with fused 1/sqrt(hd)
            scale (no max-subtraction needed: l2norm bounds |s| <= 1),
            causal masking = fp16 0/1 multiply, denominators via all-ones
            matmul, AV with v as lhsT (fp16), yT normalized by reciprocal
  out     = Wproj^T consumes yT directly (bf16), written feature-major
Host transposes the [C, 1024] result and scatters rows p::2 of batch b.
"""

import numpy as np
import ml_dtypes

import concourse.bacc as bacc
import concourse.mybir as mybir
import concourse.tile as tile
from concourse.bass_utils import run_bass_kernel_spmd

B, T, C = 4, 2048, 2048
NH, NKV, HD = 16, 4, 128
KV = 512            # k (and v) projection width
P = 128
SCALE = 1.0 / float(np.sqrt(HD))
N_CORES = 8
W_SCALE = 64.0      # host pre-scale on Wq/Wk so fp8 values are normal-range

F32 = mybir.dt.float32
F32R = mybir.dt.float32r
BF16 = mybir.dt.bfloat16
FP16 = mybir.dt.float16
F8 = mybir.dt.float8e4
Exp = mybir.ActivationFunctionType.Exp
Sqrt = mybir.ActivationFunctionType.Sqrt
MUL = mybir.AluOpType.mult
DR = mybir.MatmulPerfMode.DoubleRow

NST = 4             # four 512-row T strips
STS = 4             # 128-row blocks per strip
NCC = 16            # C contraction chunks of 128
NCP = NCC // 2      # 8 DoubleRow contraction pairs
NM_K = KV // P      # 4  (kdim / vdim output chunks)
NM_Q = C // P       # 16 (qdim output chunks)
LQ = T // 2         # 1024 local query rows per core
SS_W = 512          # local queries per attention strip


def build():
    nc = bacc.Bacc("TRN2", target_bir_lowering=False, debug=False,
                   num_devices=N_CORES)
    xb = nc.declare_dram_parameter("xb", [T, C], BF16, isOutput=False)
    # DoubleRow-interleaved fp8 weights: [p, m, j, i, f] =
    # W[(2j+i)*128+p, m*128+f] * W_SCALE
    wq8 = nc.declare_dram_parameter("wq8", [P, NM_Q, NCP, 2, P], F8,
                                    isOutput=False)
    wk8 = nc.declare_dram_parameter("wk8", [P, NM_K, NCP, 2, P], F8,
                                    isOutput=False)
    wv = nc.declare_dram_parameter("wv", [C, KV], BF16, isOutput=False)
    wp = nc.declare_dram_parameter("wp", [C, C], BF16, isOutput=False)
    masks = nc.declare_dram_parameter("masks", [8, P, SS_W], FP16,
                                      isOutput=False)
    ident_in = nc.declare_dram_parameter("ident", [P, P], BF16, isOutput=False)
    ones_in = nc.declare_dram_parameter("onesb", [P, P], BF16, isOutput=False)
    onesh_in = nc.declare_dram_parameter("onesh", [P, P], FP16, isOutput=False)
    out = nc.declare_dram_parameter("out", [C, LQ], F32, isOutput=True)

    with tile.TileContext(nc) as tc:
        with (
            tc.tile_pool(name="cst", bufs=1) as cst,
            tc.tile_pool(name="qT", bufs=1) as p_qT,
            tc.tile_pool(name="kT", bufs=1) as p_kT,
            tc.tile_pool(name="vsb", bufs=1) as p_v,
            tc.tile_pool(name="xq8", bufs=1) as p_xq8,
        ):
            ident = cst.tile([P, P], BF16)
            nc.sync.dma_start(ident[:], ident_in[:])
            ones_bf = cst.tile([P, P], BF16)
            nc.sync.dma_start(ones_bf[:], ones_in[:])
            ones_h = cst.tile([P, P], FP16)
            nc.sync.dma_start(ones_h[:], onesh_in[:])

            qT_sb = p_qT.tile([P, NM_Q, LQ], BF16)       # 32 KB/part
            kT_sb = p_kT.tile([P, NM_K, T], BF16)        # 16 KB/part
            v_sb = p_v.tile([P, NCC, KV], FP16)          # 16 KB/part
            xq8_sb = p_xq8.tile([P, NCC, LQ], F8)        # 16 KB/part

            # ------------- Phases K (kv proj) and Q (q proj) -------------
            if True:
                with (
                    tc.tile_pool(name="xTb", bufs=2) as p_xTb,
                    tc.tile_pool(name="xT8", bufs=2) as p_xT8,
                    tc.tile_pool(name="vT", bufs=1) as p_vT,
                    tc.tile_pool(name="wt", bufs=4) as p_wt,
                    tc.tile_pool(name="xin", bufs=4) as p_xin,
                    tc.tile_pool(name="nrm", bufs=2) as p_nrm,
                    tc.tile_pool(name="sq", bufs=2) as p_sq,
                    tc.tile_pool(name="pt", bufs=3, space="PSUM") as ps_t,
                    tc.tile_pool(name="acc", bufs=3, space="PSUM") as ps_a,
                    tc.tile_pool(name="pssq", bufs=2, space="PSUM") as ps_q,
                ):
                    def transpose_strip(st):
                        t0 = st * 512
                        # xT[:, cc, :] = x[t0:t0+512, cc*128..].T  (bf16)
                        xT = p_xTb.tile([P, NCC, 512], BF16, tag="xT")
                        for ts in range(STS):
                            x_sb = p_xin.tile([P, C], BF16, tag="xin")
                            nc.sync.dma_start(
                                x_sb[:],
                                xb[t0 + ts * P: t0 + (ts + 1) * P, :])
                            for c4 in range(4):
                                pt = ps_t.tile([P, 4, P], BF16, tag="pt")
                                for j in range(4):
                                    cc = c4 * 4 + j
                                    nc.tensor.transpose(
                                        pt[:, j, :],
                                        x_sb[:, cc * P:(cc + 1) * P],
                                        ident[:])
                                nc.vector.tensor_copy(
                                    xT[:, c4 * 4:(c4 + 1) * 4,
                                       ts * P:(ts + 1) * P],
                                    pt[:])
                        # fp8 copy of the strip + strided query extraction
                        xT8 = p_xT8.tile([P, NCC, 512], F8, tag="xT8")
                        nc.scalar.copy(out=xT8[:], in_=xT[:])
                        nc.gpsimd.tensor_copy(
                            xq8_sb[:, :, st * 256:(st + 1) * 256],
                            xT8[:, :, 0::2])
                        return xT, xT8

                    def kv_proj(st, xT, xT8):
                        t0 = st * 512
                        tsl = slice(t0, t0 + 512)
                        # ---- kT projection (fp8 DoubleRow) + l2norm(k) ----
                        ssqk = ps_q.tile([P, 512], F32, tag="ssq")
                        for m in range(NM_K):
                            wk_t = p_wt.tile([P, NCP, 2, P], F8, tag="wt")
                            nc.sync.dma_start(wk_t[:], wk8[:, m])
                            pk = ps_a.tile([P, 512], F32, tag="acc")
                            for j in range(NCP):
                                nc.tensor.matmul(
                                    pk[:], wk_t[:, j, :, :],
                                    xT8[:, 2 * j:2 * j + 2, :],
                                    start=(j == 0), stop=(j == NCP - 1),
                                    perf_mode=DR)
                            nc.scalar.copy(out=kT_sb[:, m, tsl], in_=pk[:])
                            sq = p_sq.tile([P, 512], BF16, tag="sq")
                            nc.vector.tensor_tensor(
                                sq[:], kT_sb[:, m, tsl], kT_sb[:, m, tsl],
                                MUL)
                            nc.tensor.matmul(
                                ssqk[:], ones_bf[:], sq[:],
                                start=(m == 0), stop=(m == NM_K - 1))
                        nrm = p_nrm.tile([P, 512], F32, tag="nrm")
                        nc.scalar.activation(nrm[:], ssqk[:], Sqrt)
                        rk = p_nrm.tile([P, 512], F32, tag="rk")
                        nc.vector.reciprocal_approx_fast(rk[:], nrm[:])
                        rk16 = p_sq.tile([P, 512], BF16, tag="rk16")
                        nc.vector.tensor_copy(rk16[:], rk[:])
                        nc.vector.tensor_tensor(
                            kT_sb[:, :, tsl], kT_sb[:, :, tsl],
                            rk16[:, None, :].to_broadcast([P, NM_K, 512]),
                            MUL)

                        # ---- vT projection (bf16), then transpose to v ----
                        vT = p_vT.tile([P, NM_K, 512], BF16, tag="vT")
                        for m in range(NM_K):
                            wv_t = p_wt.tile([P, NCC, P], BF16, tag="wtv")
                            nc.sync.dma_start(
                                wv_t[:],
                                wv[:, m * P:(m + 1) * P].rearrange(
                                    "(cc p) f -> p cc f", p=P))
                            pv = ps_a.tile([P, 512], F32, tag="acc")
                            for cc in range(NCC):
                                nc.tensor.matmul(
                                    pv[:], wv_t[:, cc, :], xT[:, cc, :],
                                    start=(cc == 0), stop=(cc == NCC - 1))
                            nc.scalar.copy(out=vT[:, m, :], in_=pv[:])
                        for m in range(NM_K):
                            pt = ps_t.tile([P, 4, P], BF16, tag="pt")
                            for j in range(4):
                                nc.tensor.transpose(
                                    pt[:, j, :], vT[:, m, j * P:(j + 1) * P],
                                    ident[:])
                            for j in range(4):
                                nc.vector.tensor_copy(
                                    v_sb[:, st * 4 + j, m * P:(m + 1) * P],
                                    pt[:, j, :])

                    # software pipeline: strip st+1's transposes issue on PE
                    # before strip st's projections, so the PE never stalls
                    # on the fp8 cast / extraction of the strip it is about
                    # to project
                    prev = None
                    for st in range(NST):
                        cur = transpose_strip(st)
                        if prev is not None:
                            kv_proj(st - 1, *prev)
                        prev = cur
                    kv_proj(NST - 1, *prev)

                # ---- Phase Q: fp8 DoubleRow q projection + l2norm ----
                with (
                    tc.tile_pool(name="wtq", bufs=4) as p_wtq,
                    tc.tile_pool(name="nrmq", bufs=2) as p_nrmq,
                    tc.tile_pool(name="sqq", bufs=2) as p_sqq,
                    tc.tile_pool(name="accq", bufs=3, space="PSUM") as ps_aq,
                    tc.tile_pool(name="ssqq", bufs=2, space="PSUM") as ps_qq,
                ):
                    ssq = [ps_qq.tile([P, 512], F32, tag="ssq",
                                      name=f"ssq{_i}")
                           for _i in range(2)]
                    for m in range(NM_Q):
                        wq_t = p_wtq.tile([P, NCP, 2, P], F8, tag="wt")
                        nc.sync.dma_start(wq_t[:], wq8[:, m])
                        for sp in range(2):
                            qsl = slice(sp * 512, (sp + 1) * 512)
                            pq = ps_aq.tile([P, 512], F32, tag="acc")
                            for j in range(NCP):
                                nc.tensor.matmul(
                                    pq[:], wq_t[:, j, :, :],
                                    xq8_sb[:, 2 * j:2 * j + 2, qsl],
                                    start=(j == 0), stop=(j == NCP - 1),
                                    perf_mode=DR)
                            nc.scalar.copy(out=qT_sb[:, m, qsl], in_=pq[:])
                            sq = p_sqq.tile([P, 512], BF16, tag="sq")
                            nc.vector.tensor_tensor(
                                sq[:], qT_sb[:, m, qsl], qT_sb[:, m, qsl],
                                MUL)
                            nc.tensor.matmul(
                                ssq[sp][:], ones_bf[:], sq[:],
                                start=(m == 0), stop=(m == NM_Q - 1))
                    for sp in range(2):
                        qsl = slice(sp * 512, (sp + 1) * 512)
                        nrm = p_nrmq.tile([P, 512], F32, tag="nrm")
                        nc.scalar.activation(nrm[:], ssq[sp][:], Sqrt)
                        rq = p_nrmq.tile([P, 512], F32, tag="rq")
                        nc.vector.reciprocal_approx_fast(rq[:], nrm[:])
                        rq16 = p_sqq.tile([P, 512], BF16, tag="rq16")
                        nc.vector.tensor_copy(rq16[:], rq[:])
                        nc.vector.tensor_tensor(
                            qT_sb[:, :, qsl], qT_sb[:, :, qsl],
                            rq16[:, None, :].to_broadcast([P, NM_Q, 512]),
                            MUL)

            # -------- Phase A (attention), then Phase O (out-proj) --------
            with (
                tc.tile_pool(name="e", bufs=6) as p_e,
                tc.tile_pool(name="rd", bufs=3) as p_rd,
                tc.tile_pool(name="osb", bufs=2) as p_o,
                tc.tile_pool(name="wpt", bufs=2) as p_wp,
                tc.tile_pool(name="acce", bufs=3) as p_acc,
                tc.tile_pool(name="ps_s", bufs=2, space="PSUM") as ps_s,
                tc.tile_pool(name="ps_y", bufs=3, space="PSUM") as ps_y,
                tc.tile_pool(name="ps_d", bufs=1, space="PSUM") as ps_d,
                tc.tile_pool(name="msk", bufs=1) as p_msk,
                tc.tile_pool(name="yT", bufs=2) as p_yT,
            ):
                mask_sb = p_msk.tile([P, 8, SS_W], FP16)
                nc.sync.dma_start(mask_sb[:],
                                  masks.rearrange("j p f -> p j f"))
                for ss in range(2):
                    nk = 8 * (ss + 1)
                    yT = p_yT.tile([P, NH, SS_W], BF16, tag="yT")
                    lsl = slice(ss * SS_W, (ss + 1) * SS_W)
                    for h in range(NH):
                        g = h // 4
                        py = ps_y.tile([P, SS_W], F32, tag="y")
                        acc = p_acc.tile([P, 2, SS_W], FP16, tag="acc")
                        for kc2 in range(nk // 2):
                            psc = ps_s.tile([P, 2, SS_W], F32, tag="s")
                            for i in range(2):
                                kc = 2 * kc2 + i
                                nc.tensor.matmul(
                                    psc[:, i, :],
                                    kT_sb[:, g, kc * P:(kc + 1) * P],
                                    qT_sb[:, h, lsl],
                                    start=True, stop=True)
                            e = p_e.tile([P, 2, SS_W], FP16, tag="e")
                            nc.scalar.activation(e[:], psc[:], Exp,
                                                 scale=SCALE)
                            j0 = 2 * kc2 - 8 * ss
                            if j0 >= 0:  # partial (diagonal-band) pair
                                nc.vector.tensor_tensor(
                                    e[:], e[:], mask_sb[:, j0:j0 + 2, :],
                                    MUL)
                            # fp16 running sum of exp chunks (for the
                            # denominators; fp16 keeps the sum accurate);
                            # acc keeps two parallel partial sums, folded
                            # once at the end
                            if kc2 == 0:
                                nc.vector.tensor_copy(acc[:], e[:])
                            else:
                                nc.vector.tensor_tensor(
                                    acc[:], acc[:], e[:],
                                    mybir.AluOpType.add)
                            for i in range(2):
                                kc = 2 * kc2 + i
                                nc.tensor.matmul(
                                    py[:],
                                    v_sb[:, kc, g * P:(g + 1) * P],
                                    e[:, i, :],
                                    start=(kc == 0), stop=(kc == nk - 1))
                        accf = p_acc.tile([P, SS_W], FP16, tag="accf")
                        nc.vector.tensor_tensor(
                            accf[:], acc[:, 0, :], acc[:, 1, :],
                            mybir.AluOpType.add)
                        pden = ps_d.tile([P, SS_W], F32, tag="d")
                        nc.tensor.matmul(pden[:], ones_h[:], accf[:],
                                         start=True, stop=True)
                        rden = p_rd.tile([P, SS_W], F32, tag="rd")
                        nc.vector.reciprocal_approx_fast(rden[:], pden[:])
                        nc.vector.tensor_tensor(yT[:, h, :], py[:], rden[:],
                                                MUL)

                    # out-projection for this strip, right after its
                    # attention so it can fill PE gaps of the next phase
                    for og in range(NM_Q // 4):
                        wp_t = p_wp.tile([P, NH, 4 * P], BF16, tag="wpt")
                        nc.sync.dma_start(
                            wp_t[:],
                            wp[:, og * 4 * P:(og + 1) * 4 * P].rearrange(
                                "(hh p) f -> p hh f", p=P))
                        o_sb = p_o.tile([P, 4, SS_W], F32, tag="o")
                        for j in range(4):
                            po = ps_s.tile([P, SS_W], F32, tag="s")
                            for hh in range(NH):
                                nc.tensor.matmul(
                                    po[:],
                                    wp_t[:, hh, j * P:(j + 1) * P],
                                    yT[:, hh, :],
                                    start=(hh == 0), stop=(hh == NH - 1))
                            nc.vector.tensor_copy(o_sb[:, j, :], po[:])
                        nc.sync.dma_start(
                            out.rearrange("(og j p) q -> p og j q", p=P,
                                          j=4)[:, og, :, lsl],
                            o_sb[:])

    nc.compile()
    return nc


_NC = None


def _get_nc():
    global _NC
    if _NC is None:
        _NC = build()
    return _NC


def _make_masks(p: int) -> np.ndarray:
    j = np.arange(8)[:, None, None]
    k = np.arange(P)[None, :, None]
    q = np.arange(SS_W)[None, None, :]
    if p == 0:
        valid = (2 * q) >= (128 * j + k)
    else:
        # odd cores see pair-swapped rows: key at in-chunk position k is
        # global row 128*kc + (k ^ 1); queries are odd rows 2q+1
        valid = (2 * q + 1) >= (128 * j + (k ^ 1))
    return valid.astype(np.float16)


def _dr_weights(w: np.ndarray, nm: int) -> np.ndarray:
    # [p, m, j, i, f] = w[(2j+i)*128+p, m*128+f] * W_SCALE, cast fp8e4m3
    cin, cout = w.shape
    w5 = (w * W_SCALE).reshape(NCP, 2, P, nm, P).transpose(2, 3, 0, 1, 4)
    return np.ascontiguousarray(w5.astype(ml_dtypes.float8_e4m3fn))


def kernel(x, Wq, Wkv, Wproj):
    x = np.asarray(x, dtype=np.float32)
    Wq = np.asarray(Wq, dtype=np.float32)
    Wkv = np.asarray(Wkv, dtype=np.float32)
    Wproj = np.asarray(Wproj, dtype=np.float32)

    wq8 = _dr_weights(Wq, NM_Q)
    wk8 = _dr_weights(Wkv[:, :KV], NM_K)
    wv16 = np.ascontiguousarray(Wkv[:, KV:]).astype(ml_dtypes.bfloat16)
    wp16 = Wproj.astype(ml_dtypes.bfloat16)
    ident = np.eye(P, dtype=np.float32).astype(ml_dtypes.bfloat16)
    onesb = np.ones((P, P), dtype=ml_dtypes.bfloat16)
    onesh = np.ones((P, P), dtype=np.float16)
    masks_by_p = [_make_masks(0), _make_masks(1)]

    in_maps = []
    for c in range(N_CORES):
        b, p = c // 2, c % 2
        if p == 0:
            xb_c = x[b]
        else:
            # pair-swap rows (2i <-> 2i+1): the program's fixed "rows 0::2"
            # query gather then selects the odd rows, every key row is still
            # present, and the swapped in-chunk key order is absorbed into
            # this core's mask data.
            xb_c = x[b].reshape(T // 2, 2, C)[:, ::-1, :].reshape(T, C)
        xb_c = np.ascontiguousarray(xb_c.astype(ml_dtypes.bfloat16))
        in_maps.append({
            "xb": xb_c,
            "wq8": wq8, "wk8": wk8, "wv": wv16, "wp": wp16,
            "masks": masks_by_p[p],
            "ident": ident, "onesb": onesb, "onesh": onesh,
        })

    nc = _get_nc()
    res = run_bass_kernel_spmd(nc, in_maps, list(range(N_CORES)),
                               trace=False)

    result = np.empty((B, T, C), dtype=np.float32)
    for c in range(N_CORES):
        b, p = c // 2, c % 2
        result[b, p::2, :] = res.results[c]["out"].T
    return result


# revision 11
# speedup vs baseline: 1.0045x; 1.0045x over previous
"""Causal GQA self-attention (B=4, T=2048, C=2048, 16 heads / 4 kv-heads,
l2-normalized q,k) on 8 Trainium2 NeuronCores.

Sharding: core pair (2b, 2b+1) handles batch b. Within a pair, queries are
split stride-2 by row parity (parity p takes rows p::2), which balances the
causal work and keeps the compiled program identical on all 8 cores. All
per-core differences are data: odd-parity cores receive x with adjacent
rows pair-swapped, so the program's fixed `rows 0::2` query gather selects
the odd rows while every key row remains present; the swapped in-chunk key
order is absorbed into that core's causal-mask tiles (a kernel input).

Per core:
  xT      = PE-transpose(x_b) in bf16, plus an fp8(e4m3) copy
  q,k     = fp8 DoubleRow-mode projections (2 contraction tiles per pass,
            0.5 cyc/row); weights pre-scaled by 64 on host to escape fp8
            subnormals -- the subsequent l2norm makes q,k scale-invariant
  v       = bf16 projection of full T, re-transposed on PE
  q rows  = this core's 1024 query rows extracted strided from the fp8 xT
            into SBUF (no DRAM bounce)
  attn    = scores^T layout sT[k,q] (bf16), exp on ACT with fused 1/sqrt(hd)
            scale (no max-subtraction needed: l2norm bounds |s| <= 1),
            causal masking = fp16 0/1 multiply, denominators via all-ones
            matmul, AV with v as lhsT (fp16), yT normalized by reciprocal
  out     = Wproj^T consumes yT directly (bf16), written feature-major
Host transposes the [C, 1024] result and scatters rows p::2 of batch b.
"""

import numpy as np
import ml_dtypes

import concourse.bacc as bacc
import concourse.mybir as mybir
import concourse.tile as tile
from concourse.bass_utils import run_bass_kernel_spmd

B, T, C = 4, 2048, 2048
NH, NKV, HD = 16, 4, 128
KV = 512            # k (and v) projection width
P = 128
SCALE = 1.0 / float(np.sqrt(HD))
N_CORES = 8
W_SCALE = 64.0      # host pre-scale on Wq/Wk so fp8 values are normal-range

F32 = mybir.dt.float32
F32R = mybir.dt.float32r
BF16 = mybir.dt.bfloat16
FP16 = mybir.dt.float16
F8 = mybir.dt.float8e4
Exp = mybir.ActivationFunctionType.Exp
Sqrt = mybir.ActivationFunctionType.Sqrt
MUL = mybir.AluOpType.mult
DR = mybir.MatmulPerfMode.DoubleRow

NST = 4             # four 512-row T strips
STS = 4             # 128-row blocks per strip
NCC = 16            # C contraction chunks of 128
NCP = NCC // 2      # 8 DoubleRow contraction pairs
NM_K = KV // P      # 4  (kdim / vdim output chunks)
NM_Q = C // P       # 16 (qdim output chunks)
LQ = T // 2         # 1024 local query rows per core
SS_W = 512          # local queries per attention strip


def build():
    nc = bacc.Bacc("TRN2", target_bir_lowering=False, debug=False,
                   num_devices=N_CORES)
    xb = nc.declare_dram_parameter("xb", [T, C], BF16, isOutput=False)
    # DoubleRow-interleaved fp8 weights: [p, m, j, i, f] =
    # W[(2j+i)*128+p, m*128+f] * W_SCALE
    wq8 = nc.declare_dram_parameter("wq8", [P, NM_Q, NCP, 2, P], F8,
                                    isOutput=False)
    wk8 = nc.declare_dram_parameter("wk8", [P, NM_K, NCP, 2, P], F8,
                                    isOutput=False)
    wv = nc.declare_dram_parameter("wv", [C, KV], BF16, isOutput=False)
    wp = nc.declare_dram_parameter("wp", [C, C], BF16, isOutput=False)
    masks = nc.declare_dram_parameter("masks", [4, P, 256], FP16,
                                      isOutput=False)
    ident_in = nc.declare_dram_parameter("ident", [P, P], BF16, isOutput=False)
    ones_in = nc.declare_dram_parameter("onesb", [P, P], BF16, isOutput=False)
    onesh_in = nc.declare_dram_parameter("onesh", [P, P], FP16, isOutput=False)
    out = nc.declare_dram_parameter("out", [C, LQ], F32, isOutput=True)

    with tile.TileContext(nc) as tc:
        with (
            tc.tile_pool(name="cst", bufs=1) as cst,
            tc.tile_pool(name="qT", bufs=1) as p_qT,
            tc.tile_pool(name="kT", bufs=1) as p_kT,
            tc.tile_pool(name="vsb", bufs=1) as p_v,
            tc.tile_pool(name="xq8", bufs=1) as p_xq8,
        ):
            ident = cst.tile([P, P], BF16)
            nc.sync.dma_start(ident[:], ident_in[:])
            ones_bf = cst.tile([P, P], BF16)
            nc.sync.dma_start(ones_bf[:], ones_in[:])
            ones_h = cst.tile([P, P], FP16)
            nc.sync.dma_start(ones_h[:], onesh_in[:])

            qT_sb = p_qT.tile([P, NM_Q, LQ], BF16)       # 32 KB/part
            kT_sb = p_kT.tile([P, NM_K, T], BF16)        # 16 KB/part
            v_sb = p_v.tile([P, NCC, KV], FP16)          # 16 KB/part
            xq8_sb = p_xq8.tile([P, NCC, LQ], F8)        # 16 KB/part

            # ------------- Phases K (kv proj) and Q (q proj) -------------
            if True:
                with (
                    tc.tile_pool(name="xTb", bufs=2) as p_xTb,
                    tc.tile_pool(name="xT8", bufs=2) as p_xT8,
                    tc.tile_pool(name="vT", bufs=1) as p_vT,
                    tc.tile_pool(name="wt", bufs=4) as p_wt,
                    tc.tile_pool(name="xin", bufs=4) as p_xin,
                    tc.tile_pool(name="nrm", bufs=2) as p_nrm,
                    tc.tile_pool(name="sq", bufs=2) as p_sq,
                    tc.tile_pool(name="pt", bufs=3, space="PSUM") as ps_t,
                    tc.tile_pool(name="acc", bufs=3, space="PSUM") as ps_a,
                    tc.tile_pool(name="pssq", bufs=2, space="PSUM") as ps_q,
                ):
                    def transpose_strip(st):
                        t0 = st * 512
                        # xT[:, cc, :] = x[t0:t0+512, cc*128..].T  (bf16)
                        xT = p_xTb.tile([P, NCC, 512], BF16, tag="xT")
                        for ts in range(STS):
                            x_sb = p_xin.tile([P, C], BF16, tag="xin")
                            nc.sync.dma_start(
                                x_sb[:],
                                xb[t0 + ts * P: t0 + (ts + 1) * P, :])
                            for c4 in range(4):
                                pt = ps_t.tile([P, 4, P], BF16, tag="pt")
                                for j in range(4):
                                    cc = c4 * 4 + j
                                    nc.tensor.transpose(
                                        pt[:, j, :],
                                        x_sb[:, cc * P:(cc + 1) * P],
                                        ident[:])
                                nc.vector.tensor_copy(
                                    xT[:, c4 * 4:(c4 + 1) * 4,
                                       ts * P:(ts + 1) * P],
                                    pt[:])
                        # fp8 copy of the strip + strided query extraction
                        xT8 = p_xT8.tile([P, NCC, 512], F8, tag="xT8")
                        nc.scalar.copy(out=xT8[:], in_=xT[:])
                        nc.gpsimd.tensor_copy(
                            xq8_sb[:, :, st * 256:(st + 1) * 256],
                            xT8[:, :, 0::2])
                        return xT, xT8

                    def kv_proj(st, xT, xT8):
                        t0 = st * 512
                        tsl = slice(t0, t0 + 512)
                        # ---- kT projection (fp8 DoubleRow) + l2norm(k) ----
                        ssqk = ps_q.tile([P, 512], F32, tag="ssq")
                        for m in range(NM_K):
                            wk_t = p_wt.tile([P, NCP, 2, P], F8, tag="wt")
                            nc.sync.dma_start(wk_t[:], wk8[:, m])
                            pk = ps_a.tile([P, 512], F32, tag="acc")
                            for j in range(NCP):
                                nc.tensor.matmul(
                                    pk[:], wk_t[:, j, :, :],
                                    xT8[:, 2 * j:2 * j + 2, :],
                                    start=(j == 0), stop=(j == NCP - 1),
                                    perf_mode=DR)
                            nc.scalar.copy(out=kT_sb[:, m, tsl], in_=pk[:])
                            sq = p_sq.tile([P, 512], BF16, tag="sq")
                            nc.vector.tensor_tensor(
                                sq[:], kT_sb[:, m, tsl], kT_sb[:, m, tsl],
                                MUL)
                            nc.tensor.matmul(
                                ssqk[:], ones_bf[:], sq[:],
                                start=(m == 0), stop=(m == NM_K - 1))
                        nrm = p_nrm.tile([P, 512], F32, tag="nrm")
                        nc.scalar.activation(nrm[:], ssqk[:], Sqrt)
                        rk = p_nrm.tile([P, 512], F32, tag="rk")
                        nc.vector.reciprocal_approx_fast(rk[:], nrm[:])
                        rk16 = p_sq.tile([P, 512], BF16, tag="rk16")
                        nc.vector.tensor_copy(rk16[:], rk[:])
                        nc.vector.tensor_tensor(
                            kT_sb[:, :, tsl], kT_sb[:, :, tsl],
                            rk16[:, None, :].to_broadcast([P, NM_K, 512]),
                            MUL)

                        # ---- vT projection (bf16), then transpose to v ----
                        vT = p_vT.tile([P, NM_K, 512], BF16, tag="vT")
                        for m in range(NM_K):
                            wv_t = p_wt.tile([P, NCC, P], BF16, tag="wtv")
                            nc.sync.dma_start(
                                wv_t[:],
                                wv[:, m * P:(m + 1) * P].rearrange(
                                    "(cc p) f -> p cc f", p=P))
                            pv = ps_a.tile([P, 512], F32, tag="acc")
                            for cc in range(NCC):
                                nc.tensor.matmul(
                                    pv[:], wv_t[:, cc, :], xT[:, cc, :],
                                    start=(cc == 0), stop=(cc == NCC - 1))
                            nc.scalar.copy(out=vT[:, m, :], in_=pv[:])
                        for m in range(NM_K):
                            pt = ps_t.tile([P, 4, P], BF16, tag="pt")
                            for j in range(4):
                                nc.tensor.transpose(
                                    pt[:, j, :], vT[:, m, j * P:(j + 1) * P],
                                    ident[:])
                            for j in range(4):
                                nc.vector.tensor_copy(
                                    v_sb[:, st * 4 + j, m * P:(m + 1) * P],
                                    pt[:, j, :])

                    # software pipeline: strip st+1's transposes issue on PE
                    # before strip st's projections, so the PE never stalls
                    # on the fp8 cast / extraction of the strip it is about
                    # to project
                    prev = None
                    for st in range(NST):
                        cur = transpose_strip(st)
                        if prev is not None:
                            kv_proj(st - 1, *prev)
                        prev = cur
                    kv_proj(NST - 1, *prev)

                # ---- Phase Q: fp8 DoubleRow q projection + l2norm ----
                with (
                    tc.tile_pool(name="wtq", bufs=4) as p_wtq,
                    tc.tile_pool(name="nrmq", bufs=2) as p_nrmq,
                    tc.tile_pool(name="sqq", bufs=2) as p_sqq,
                    tc.tile_pool(name="accq", bufs=3, space="PSUM") as ps_aq,
                    tc.tile_pool(name="ssqq", bufs=2, space="PSUM") as ps_qq,
                ):
                    ssq = [ps_qq.tile([P, 512], F32, tag="ssq",
                                      name=f"ssq{_i}")
                           for _i in range(2)]
                    for m in range(NM_Q):
                        wq_t = p_wtq.tile([P, NCP, 2, P], F8, tag="wt")
                        nc.sync.dma_start(wq_t[:], wq8[:, m])
                        for sp in range(2):
                            qsl = slice(sp * 512, (sp + 1) * 512)
                            pq = ps_aq.tile([P, 512], F32, tag="acc")
                            for j in range(NCP):
                                nc.tensor.matmul(
                                    pq[:], wq_t[:, j, :, :],
                                    xq8_sb[:, 2 * j:2 * j + 2, qsl],
                                    start=(j == 0), stop=(j == NCP - 1),
                                    perf_mode=DR)
                            nc.scalar.copy(out=qT_sb[:, m, qsl], in_=pq[:])
                            sq = p_sqq.tile([P, 512], BF16, tag="sq")
                            nc.vector.tensor_tensor(
                                sq[:], qT_sb[:, m, qsl], qT_sb[:, m, qsl],
                                MUL)
                            nc.tensor.matmul(
                                ssq[sp][:], ones_bf[:], sq[:],
                                start=(m == 0), stop=(m == NM_Q - 1))
                    for sp in range(2):
                        qsl = slice(sp * 512, (sp + 1) * 512)
                        nrm = p_nrmq.tile([P, 512], F32, tag="nrm")
                        nc.scalar.activation(nrm[:], ssq[sp][:], Sqrt)
                        rq = p_nrmq.tile([P, 512], F32, tag="rq")
                        nc.vector.reciprocal_approx_fast(rq[:], nrm[:])
                        rq16 = p_sqq.tile([P, 512], BF16, tag="rq16")
                        nc.vector.tensor_copy(rq16[:], rq[:])
                        nc.vector.tensor_tensor(
                            qT_sb[:, :, qsl], qT_sb[:, :, qsl],
                            rq16[:, None, :].to_broadcast([P, NM_Q, 512]),
                            MUL)

            # -------- Phase A (attention), then Phase O (out-proj) --------
            with (
                tc.tile_pool(name="e", bufs=6) as p_e,
                tc.tile_pool(name="rd", bufs=3) as p_rd,
                tc.tile_pool(name="osb", bufs=2) as p_o,
                tc.tile_pool(name="wpt", bufs=2) as p_wp,
                tc.tile_pool(name="acce", bufs=3) as p_acc,
                tc.tile_pool(name="ps_s", bufs=2, space="PSUM") as ps_s,
                tc.tile_pool(name="ps_y", bufs=3, space="PSUM") as ps_y,
                tc.tile_pool(name="ps_d", bufs=1, space="PSUM") as ps_d,
                tc.tile_pool(name="msk", bufs=1) as p_msk,
                tc.tile_pool(name="yT", bufs=2) as p_yT,
            ):
                mask_sb = p_msk.tile([P, 4, 256], FP16)
                nc.sync.dma_start(mask_sb[:],
                                  masks.rearrange("j p f -> p j f"))
                for ss in range(2):
                    yT = p_yT.tile([P, NH, SS_W], BF16, tag="yT")
                    lsl = slice(ss * SS_W, (ss + 1) * SS_W)
                    # two 256-query causal sub-strips per 512-query out-proj
                    # block: sub-strip ssf needs only 4*(ssf+1) key chunks,
                    # trimming ~17% of scores/AV/exp/mask area vs 512-query
                    # granularity. The diagonal-band masks are the same 4
                    # chunk patterns for every sub-strip.
                    for sq in range(2):
                        ssf = 2 * ss + sq
                        nk = 4 * (ssf + 1)
                        qsl = slice(ssf * 256, (ssf + 1) * 256)
                        ysl = slice(sq * 256, (sq + 1) * 256)
                        for h in range(NH):
                            g = h // 4
                            py = ps_y.tile([P, 256], F32, tag="y")
                            acc = p_acc.tile([P, 2, 256], FP16, tag="acc")
                            for kc2 in range(nk // 2):
                                psc = ps_s.tile([P, 2, 256], F32, tag="s")
                                for i in range(2):
                                    kc = 2 * kc2 + i
                                    nc.tensor.matmul(
                                        psc[:, i, :],
                                        kT_sb[:, g, kc * P:(kc + 1) * P],
                                        qT_sb[:, h, qsl],
                                        start=True, stop=True)
                                e = p_e.tile([P, 2, 256], FP16, tag="e")
                                nc.scalar.activation(e[:], psc[:], Exp,
                                                     scale=SCALE)
                                j0 = 2 * kc2 - 4 * ssf
                                if j0 >= 0:  # partial (diagonal-band) pair
                                    nc.vector.tensor_tensor(
                                        e[:], e[:],
                                        mask_sb[:, j0:j0 + 2, :], MUL)
                                # fp16 running sum of exp chunks (for the
                                # denominators); acc keeps two parallel
                                # partial sums, folded once at the end
                                if kc2 == 0:
                                    nc.vector.tensor_copy(acc[:], e[:])
                                else:
                                    nc.vector.tensor_tensor(
                                        acc[:], acc[:], e[:],
                                        mybir.AluOpType.add)
                                for i in range(2):
                                    kc = 2 * kc2 + i
                                    nc.tensor.matmul(
                                        py[:],
                                        v_sb[:, kc, g * P:(g + 1) * P],
                                        e[:, i, :],
                                        start=(kc == 0),
                                        stop=(kc == nk - 1))
                            accf = p_acc.tile([P, 256], FP16, tag="accf")
                            nc.vector.tensor_tensor(
                                accf[:], acc[:, 0, :], acc[:, 1, :],
                                mybir.AluOpType.add)
                            pden = ps_d.tile([P, 256], F32, tag="d")
                            nc.tensor.matmul(pden[:], ones_h[:], accf[:],
                                             start=True, stop=True)
                            rden = p_rd.tile([P, 256], F32, tag="rd")
                            nc.vector.reciprocal_approx_fast(rden[:],
                                                             pden[:])
                            nc.vector.tensor_tensor(yT[:, h, ysl], py[:],
                                                    rden[:], MUL)

                    # out-projection for this strip, right after its
                    # attention so it can fill PE gaps of the next phase
                    for og in range(NM_Q // 4):
                        wp_t = p_wp.tile([P, NH, 4 * P], BF16, tag="wpt")
                        nc.sync.dma_start(
                            wp_t[:],
                            wp[:, og * 4 * P:(og + 1) * 4 * P].rearrange(
                                "(hh p) f -> p hh f", p=P))
                        o_sb = p_o.tile([P, 4, SS_W], F32, tag="o")
                        for j in range(4):
                            po = ps_s.tile([P, SS_W], F32, tag="s")
                            for hh in range(NH):
                                nc.tensor.matmul(
                                    po[:],
                                    wp_t[:, hh, j * P:(j + 1) * P],
                                    yT[:, hh, :],
                                    start=(hh == 0), stop=(hh == NH - 1))
                            nc.vector.tensor_copy(o_sb[:, j, :], po[:])
                        nc.sync.dma_start(
                            out.rearrange("(og j p) q -> p og j q", p=P,
                                          j=4)[:, og, :, lsl],
                            o_sb[:])

    nc.compile()
    return nc


_NC = None


def _get_nc():
    global _NC
    if _NC is None:
        _NC = build()
    return _NC


def _make_masks(p: int) -> np.ndarray:
    # diagonal-band chunk masks for a 256-local-query sub-strip; the strip
    # offset cancels (512*ssf on both sides), so 4 patterns serve all strips
    j = np.arange(4)[:, None, None]
    k = np.arange(P)[None, :, None]
    q = np.arange(256)[None, None, :]
    if p == 0:
        valid = (2 * q) >= (128 * j + k)
    else:
        # odd cores see pair-swapped rows: key at in-chunk position k is
        # global row 128*kc + (k ^ 1); queries are odd rows 2q+1
        valid = (2 * q + 1) >= (128 * j + (k ^ 1))
    return valid.astype(np.float16)


def _dr_weights(w: np.ndarray, nm: int) -> np.ndarray:
    # [p, m, j, i, f] = w[(2j+i)*128+p, m*128+f] * W_SCALE, cast fp8e4m3
    cin, cout = w.shape
    w5 = (w * W_SCALE).reshape(NCP, 2, P, nm, P).transpose(2, 3, 0, 1, 4)
    return np.ascontiguousarray(w5.astype(ml_dtypes.float8_e4m3fn))


def kernel(x, Wq, Wkv, Wproj):
    x = np.asarray(x, dtype=np.float32)
    Wq = np.asarray(Wq, dtype=np.float32)
    Wkv = np.asarray(Wkv, dtype=np.float32)
    Wproj = np.asarray(Wproj, dtype=np.float32)

    wq8 = _dr_weights(Wq, NM_Q)
    wk8 = _dr_weights(Wkv[:, :KV], NM_K)
    wv16 = np.ascontiguousarray(Wkv[:, KV:]).astype(ml_dtypes.bfloat16)
    wp16 = Wproj.astype(ml_dtypes.bfloat16)
    ident = np.eye(P, dtype=np.float32).astype(ml_dtypes.bfloat16)
    onesb = np.ones((P, P), dtype=ml_dtypes.bfloat16)
    onesh = np.ones((P, P), dtype=np.float16)
    masks_by_p = [_make_masks(0), _make_masks(1)]

    in_maps = []
    for c in range(N_CORES):
        b, p = c // 2, c % 2
        if p == 0:
            xb_c = x[b]
        else:
            # pair-swap rows (2i <-> 2i+1): the program's fixed "rows 0::2"
            # query gather then selects the odd rows, every key row is still
            # present, and the swapped in-chunk key order is absorbed into
            # this core's mask data.
            xb_c = x[b].reshape(T // 2, 2, C)[:, ::-1, :].reshape(T, C)
        xb_c = np.ascontiguousarray(xb_c.astype(ml_dtypes.bfloat16))
        in_maps.append({
            "xb": xb_c,
            "wq8": wq8, "wk8": wk8, "wv": wv16, "wp": wp16,
            "masks": masks_by_p[p],
            "ident": ident, "onesb": onesb, "onesh": onesh,
        })

    nc = _get_nc()
    res = run_bass_kernel_spmd(nc, in_maps, list(range(N_CORES)),
                               trace=False)

    result = np.empty((B, T, C), dtype=np.float32)
    for c in range(N_CORES):
        b, p = c // 2, c % 2
        result[b, p::2, :] = res.results[c]["out"].T
    return result
